# revision 24
# baseline (speedup 1.0000x reference)
"""Trainium2 Bass kernel for MABClean (cross-attention block with SetNorm).

Sharding: 8 cores = (batch b in 0..3) x (query-half in 0..1). Each core:
  - gets X[b] (rows permuted so its query half comes first) and Y[b], both
    transposed to feature-major [256, 2048] layout,
  - computes SetNorm stats of X/Y on-device, Q for its 1024 queries, full
    K/V, attention (per-head scores via 32-row partition-sliced bf16
    matmuls, exp on ACT emitting fp8, AV as fp8 DoubleRow matmuls packing
    kc-pairs, with a ones-column producing softmax denominators),
    O/residual,
  - AllReduces the final SetNorm (sum, sumsq) with its pair core (tiny
    payload, ring pre-warmed by a dummy collective), applies
    norm+relu+res projection, returns H^T half.
"""

import math

import numpy as np

import concourse.bass as bass
import concourse.tile as tile
from concourse import bacc, mybir
from concourse.bass_utils import run_bass_kernel_spmd

F32 = mybir.dt.float32
BF16 = mybir.dt.bfloat16
FP8 = mybir.dt.float8e4
AF = mybir.ActivationFunctionType
ALU = mybir.AluOpType
PM = mybir.MatmulPerfMode

P = 128
D = 256      # feature dim (dX = dY)
NQ = 1024    # queries per core
NK = 2048    # keys
H = 8        # heads
DH = 32      # head dim
NKC = NK // P    # 16 key chunks
NT = NKC // 2    # 8 kc-pairs for DoubleRow AV
HB = 48          # head block in VO: 32 V dims + 1 ones + 15 pad
EPS = 1e-5
WNAMES = ["WvT", "WkT", "WqT", "WoT", "WresT"]
PNAMES = ["bq", "bk", "bo", "bres", "nqw", "nqb", "nkw", "nkb",
          "n0w", "n0b"]

_CACHE = {}


def build_module():
    nc = bacc.Bacc("TRN2", target_bir_lowering=False, debug=False,
                   num_devices=8)

    # ---- DRAM I/O ----
    XT = nc.dram_tensor("XT", [D, NK], F32, kind="ExternalInput").ap()
    YT = nc.dram_tensor("YT", [D, NK], F32, kind="ExternalInput").ap()
    # all five weight matrices concatenated: [D, 5*D] bf16
    WCAT = nc.dram_tensor("WCAT", [D, 5 * D], BF16, kind="ExternalInput").ap()
    # param vectors concatenated: [D, 10] fp32
    PVEC = nc.dram_tensor("PVEC", [D, len(PNAMES)], F32,
                          kind="ExternalInput").ap()
    bv = nc.dram_tensor("bv", [D], F32, kind="ExternalInput")
    OUT = nc.dram_tensor("OUT", [D, NQ], F32, kind="ExternalOutput").ap()

    with tile.TileContext(nc) as tc:
        with (
            tc.tile_pool(name="persist", bufs=1) as pe,
            tc.tile_pool(name="work", bufs=3) as wk,
            tc.tile_pool(name="small", bufs=4) as sm,
            tc.tile_pool(name="stpool", bufs=2, space="PSUM") as stp,
            tc.tile_pool(name="opool", bufs=1, space="PSUM") as op,
            tc.tile_pool(name="dram", bufs=2, space="DRAM") as dp,
        ):
            # ---- constants / zero zones (gpsimd queue: frees DVE) ----
            ones_col = pe.tile([P, 1], F32, tag="ones_col", name="ones_col")
            nc.gpsimd.memset(ones_col[:], 1.0)
            ones_row = pe.tile([1, P], F32, tag="ones_row", name="ones_row")
            nc.gpsimd.memset(ones_row[:], 1.0)
            zero_col = pe.tile([P, 1], F32, tag="zero_col", name="zero_col")
            nc.gpsimd.memset(zero_col[:], 0.0)
            eps_t = sm.tile([1, 1], F32, tag="eps_t", name="eps_t")
            nc.vector.memset(eps_t[:], EPS)

            # VO: [P, kc-pair, kc-parity, 8 heads x 48]; 32 V dims + ones
            # col (softmax denominator) + 15 zero pad per head block.
            VO = pe.tile([P, NT, 2, H * HB], FP8, tag="VO", name="VO")
            vview = VO[:].rearrange("p t s (h e) -> p t s h e", e=HB)
            nc.gpsimd.memset(vview[:, :, :, :, 32:HB], 0.0)
            nc.gpsimd.memset(vview[:, :, :, :, 32:33], 1.0)

            # ---- load inputs (Y first: its dependency chain is longest) ----
            YTs = [pe.tile([P, NK], F32, tag=f"YT{i}", name=f"YT{i}")
                   for i in range(2)]
            XTs = [pe.tile([P, NK], F32, tag=f"XT{i}", name=f"XT{i}")
                   for i in range(2)]
            qs = [nc.sync, nc.scalar, nc.gpsimd]
            qi = 0
            for h_ in range(8):
                for i in range(2):
                    qs[qi % 3].dma_start(
                        out=YTs[i][:, h_ * 256:(h_ + 1) * 256],
                        in_=YT[i * P:(i + 1) * P, h_ * 256:(h_ + 1) * 256])
                    qi += 1
            wcat = [pe.tile([P, 5 * D], BF16, tag=f"wcat{i}", name=f"wcat{i}")
                    for i in range(2)]
            for i in range(2):
                qs[qi % 3].dma_start(out=wcat[i][:],
                                     in_=WCAT[i * P:(i + 1) * P, :])
                qi += 1
            Ws = {n: [wcat[i][:, k * D:(k + 1) * D] for i in range(2)]
                  for k, n in enumerate(WNAMES)}
            pvt = [pe.tile([P, len(PNAMES)], F32, tag=f"pv{i}", name=f"pv{i}")
                   for i in range(2)]
            for i in range(2):
                qs[qi % 3].dma_start(out=pvt[i][:],
                                     in_=PVEC[i * P:(i + 1) * P, :])
                qi += 1
            pp = {n: [pvt[i][:, k:k + 1] for i in range(2)]
                  for k, n in enumerate(PNAMES)}
            bv_bc = pe.tile([P, D], F32, tag="bv_bc", name="bv_bc")
            nc.sync.dma_start(
                out=bv_bc[:],
                in_=bass.AP(tensor=bv, offset=0, ap=[[0, P], [1, D]]))
            for h_ in range(4):
                for i in range(2):
                    qs[qi % 3].dma_start(
                        out=XTs[i][:, h_ * 512:(h_ + 1) * 512],
                        in_=XT[i * P:(i + 1) * P, h_ * 512:(h_ + 1) * 512])
                    qi += 1

            # ---- PE warm-up: keep the HAM clocked up until real work ----
            warm_sb = pe.tile([P, 512], BF16, tag="warm_sb", name="warm_sb")
            nc.vector.memset(warm_sb[:], 0.0)
            for i in range(52):
                pw = op.tile([P, 512], F32, tag="Os0", name="warm")
                nc.tensor.matmul(pw[:], lhsT=warm_sb[:, 0:P],
                                 rhs=warm_sb[:], start=True, stop=True)

            # ---- pre-warm the collective ring with a dummy AllReduce ----
            cc_sb = sm.tile([1, P], F32, tag="cc_sb", name="cc_sb")
            nc.vector.memset(cc_sb[:], 0.0)
            warm_in = dp.tile([1, P], F32)
            warm_out = dp.tile([1, P], F32)
            nc.gpsimd.dma_start(out=warm_in[:], in_=cc_sb[:])
            nc.gpsimd.collective_compute(
                "AllReduce", ALU.add,
                replica_groups=[[0, 1], [2, 3], [4, 5], [6, 7]],
                ins=[warm_in.opt()], outs=[warm_out.opt()])

            # ---- helpers ----
            def sums_of(chunks, F, tagp):
                """chunks: [128, F] fp32 tiles -> ssum [1,2] SBUF
                (sum of per-partition means, sum of per-partition ex2)."""
                nsub = F // 512
                psum_s = stp.tile([P, 2, 512], F32, tag="ST",
                                  name="pstat")[:, 0, :]
                for ci, ch in enumerate(chunks):
                    sview = ch[:].rearrange("p (n f) -> p n f", f=512)
                    st = sm.tile([P, nsub, 6], F32, tag=f"bns_{tagp}",
                                 name=f"bns_{tagp}")
                    for i in range(nsub):
                        nc.vector.bn_stats(out=st[:, i, :], in_=sview[:, i, :])
                    mv = sm.tile([P, 2], F32, tag=f"mv_{tagp}", name=f"mv_{tagp}")
                    nc.vector.bn_aggr(out=mv[:], in_=st[:])
                    ms2 = sm.tile([P, 2], F32, tag=f"ms2_{tagp}",
                                  name=f"ms2_{tagp}")
                    nc.vector.tensor_copy(out=ms2[:, 0:1], in_=mv[:, 0:1])
                    nc.vector.scalar_tensor_tensor(
                        out=ms2[:, 1:2], in0=mv[:, 0:1], scalar=mv[:, 0:1],
                        in1=mv[:, 1:2], op0=ALU.mult, op1=ALU.add)
                    nc.tensor.matmul(psum_s[0:1, 0:2], lhsT=ones_col[:],
                                     rhs=ms2[:], start=(ci == 0),
                                     stop=(ci == len(chunks) - 1))
                ssum = sm.tile([1, 2], F32, tag=f"ssum_{tagp}",
                               name=f"ssum_{tagp}")
                nc.vector.tensor_copy(out=ssum[:], in_=psum_s[0:1, 0:2])
                return ssum

            def finish_stats(ssum, nparts, tagp):
                """ssum [1,2] -> bc [128,2]: col0 -mean, col1 1/sd."""
                st2 = sm.tile([1, 2], F32, tag=f"st2_{tagp}", name=f"st2_{tagp}")
                nc.vector.tensor_scalar_mul(out=st2[:], in0=ssum[:],
                                            scalar1=1.0 / nparts)
                negvar = sm.tile([1, 1], F32, tag=f"nv_{tagp}", name=f"nv_{tagp}")
                nc.vector.scalar_tensor_tensor(
                    out=negvar[:], in0=st2[:, 0:1], scalar=st2[:, 0:1],
                    in1=st2[:, 1:2], op0=ALU.mult, op1=ALU.subtract)
                sd = sm.tile([1, 1], F32, tag=f"sd_{tagp}", name=f"sd_{tagp}")
                nc.scalar.activation(out=sd[:], in_=negvar[:], func=AF.Sqrt,
                                     bias=eps_t[:], scale=-1.0)
                inv = sm.tile([1, 2], F32, tag=f"inv_{tagp}", name=f"inv_{tagp}")
                nc.vector.reciprocal(out=inv[:, 1:2], in_=sd[:])
                nc.vector.tensor_scalar_mul(out=inv[:, 0:1], in0=st2[:, 0:1],
                                            scalar1=-1.0)
                pb = stp.tile([P, 2, 512], F32, tag="ST",
                              name=f"pb_{tagp}")[:, 0, 0:2]
                nc.tensor.matmul(pb, lhsT=ones_row[:], rhs=inv[:],
                                 start=True, stop=True)
                bc = sm.tile([P, 2], F32, tag=f"bc_{tagp}", name=f"bc_{tagp}")
                nc.vector.tensor_copy(out=bc[:], in_=pb)
                return bc

            def factors(bc, wname, bname, tagp):
                """Per-chunk scale a = w*inv, shift b = a*(-mean) + beta."""
                outs = []
                for i in range(2):
                    a = pe.tile([P, 1], F32, tag=f"a_{tagp}{i}", name=f"a_{tagp}{i}")
                    nc.vector.tensor_scalar_mul(out=a[:], in0=pp[wname][i],
                                                scalar1=bc[:, 1:2])
                    b = pe.tile([P, 1], F32, tag=f"b_{tagp}{i}", name=f"b_{tagp}{i}")
                    nc.vector.scalar_tensor_tensor(
                        out=b[:], in0=a[:], scalar=bc[:, 0:1],
                        in1=pp[bname][i], op0=ALU.mult, op1=ALU.add)
                    outs.append((a, b))
                return outs

            # ---- Y/X stats (X chain overlaps K projection), Yn, K ----
            ssY = sums_of(YTs, NK, "y")
            bcY = finish_stats(ssY, 2 * P, "y")
            fY = factors(bcY, "nkw", "nkb", "y")
            ssX = sums_of(XTs, NK, "x")
            bcX = finish_stats(ssX, 2 * P, "x")
            fX = factors(bcX, "nqw", "nqb", "x")
            YnT = []
            for i in range(2):
                t = pe.tile([P, NK], BF16, tag=f"YnT{i}", name=f"YnT{i}")
                for c4 in range(4):
                    nc.vector.tensor_scalar(
                        out=t[:, c4 * 512:(c4 + 1) * 512],
                        in0=YTs[i][:, c4 * 512:(c4 + 1) * 512],
                        scalar1=fY[i][0][:],
                        scalar2=fY[i][1][:], op0=ALU.mult, op1=ALU.add)
                YnT.append(t)
            KTs = [pe.tile([P, NK], BF16, tag=f"KT{i}", name=f"KT{i}")
                   for i in range(2)]
            pu = 0
            for fo in range(2):
                for nt in range(4):
                    pk = op.tile([P, 512], F32, tag=f"Os{pu % 4}",
                                 name="pproj")
                    pu += 1
                    for cc in range(2):
                        nc.tensor.matmul(
                            pk[:],
                            lhsT=Ws["WkT"][cc][:, fo * P:(fo + 1) * P],
                            rhs=YnT[cc][:, nt * 512:(nt + 1) * 512],
                            start=(cc == 0), stop=(cc == 1))
                    nc.vector.tensor_scalar_add(
                        out=KTs[fo][:, nt * 512:(nt + 1) * 512],
                        in0=pk[:], scalar1=pp["bk"][fo])

            # ---- Xn, Q projection ----
            XnT = []
            for i in range(2):
                t = pe.tile([P, NQ], BF16, tag=f"XnT{i}", name=f"XnT{i}")
                nc.vector.tensor_scalar(
                    out=t[:], in0=XTs[i][:, 0:NQ], scalar1=fX[i][0][:],
                    scalar2=fX[i][1][:], op0=ALU.mult, op1=ALU.add)
                XnT.append(t)
            QTs = [pe.tile([P, NQ], BF16, tag=f"QT{i}", name=f"QT{i}")
                   for i in range(2)]
            # zero-padded per-head copies: full-K matmul contracts one
            # head (216ns/call warm; K=32 sliced matmuls run 2x slower)
            QTz = [[pe.tile([P, NQ], BF16, tag=f"QTz{hg}{j}",
                            name=f"QTz{hg}{j}") for j in range(4)]
                   for hg in range(2)]
            for hg in range(2):
                for j in range(4):
                    nc.gpsimd.memset(QTz[hg][j][:], 0.0)
            for fo in range(2):
                for qt in range(2):
                    pq = op.tile([P, 512], F32, tag=f"Os{pu % 4}",
                                 name="pproj")
                    pu += 1
                    for cc in range(2):
                        nc.tensor.matmul(
                            pq[:],
                            lhsT=Ws["WqT"][cc][:, fo * P:(fo + 1) * P],
                            rhs=XnT[cc][:, qt * 512:(qt + 1) * 512],
                            start=(cc == 0), stop=(cc == 1))
                    nc.vector.tensor_scalar_add(
                        out=QTs[fo][:, qt * 512:(qt + 1) * 512],
                        in0=pq[:], scalar1=pp["bq"][fo])
                for j in range(4):
                    nc.vector.tensor_copy(
                        out=QTz[fo][j][32 * j:32 * j + 32, :],
                        in_=QTs[fo][32 * j:32 * j + 32, :])

            # ---- V projection -> VO (emitted per-kc inside phase 0,0) ----
            Yb = []
            for i in range(2):
                t = pe.tile([P, NK], BF16, tag=f"Yb{i}", name=f"Yb{i}")
                nc.vector.tensor_copy(out=t[:], in_=YTs[i][:])
                Yb.append(t)

            def emit_v(kc):
                global_u = kc
                pv = op.tile([P, 512], F32, tag=f"Os{kc % 4}",
                             name="pproj")[:, 0:D]
                for cc in range(2):
                    nc.tensor.matmul(
                        pv, lhsT=Yb[cc][:, kc * P:(kc + 1) * P],
                        rhs=Ws["WvT"][cc][:],
                        start=(cc == 0), stop=(cc == 1))
                nc.vector.tensor_add(
                    out=vview[:, kc // 2, kc % 2, :, 0:32],
                    in0=pv.rearrange("p (h e) -> p h e", e=32),
                    in1=bv_bc[:].rearrange("p (h e) -> p h e", e=32))
            for kc in range(NKC):
                emit_v(kc)

            # ---- attention ----
            OcatT = [pe.tile([P, NQ], BF16, tag=f"Ocat{i}", name=f"Ocat{i}")
                     for i in range(2)]
            H1T = [pe.tile([P, NQ], F32, tag=f"H1T{i}", name=f"H1T{i}")
                   for i in range(2)]

            def emit_phase(qt, hg, mid=None):
                """Scores -> exp -> AV for one (qt, hg); Ocat rescale."""
                Os = [op.tile([P, 512], F32, tag=f"Os{g}", name=f"Os{g}")
                      for g in range(4)]
                for t in range(NT):
                    if t == 4 and mid is not None:
                        mid()
                    for j in range(4):
                        ST = stp.tile([P, 2, 512], F32, tag="ST", name="ST")
                        ET = wk.tile([P, 2, 512], FP8, tag="ET", name="ET")
                        h = 4 * hg + j
                        for s in range(2):
                            kc = 2 * t + s
                            nc.tensor.matmul(
                                ST[:, s, :],
                                lhsT=KTs[hg][:, kc * P:(kc + 1) * P],
                                rhs=QTz[hg][j][:,
                                              qt * 512:(qt + 1) * 512],
                                start=True, stop=True)
                        nc.scalar.activation(out=ET[:], in_=ST[:],
                                             func=AF.Exp,
                                             bias=zero_col[:],
                                             scale=0.0625)
                        nc.tensor.matmul(
                            Os[j][0:HB, :],
                            lhsT=VO[:, t, :, HB * h:HB * h + HB],
                            rhs=ET[:],
                            start=(t == 0), stop=(t == NT - 1),
                            perf_mode=PM.DoubleRow)
                # free Os quickly: denominators + numerators out to SBUF
                dall = sm.tile([1, 4, 512], F32, tag="dall", name="dall")
                for j in range(4):
                    nc.vector.tensor_copy(out=dall[:, j, :],
                                          in_=Os[j][32:33, :])
                    nc.vector.tensor_copy(
                        out=OcatT[hg][32 * j:32 * j + 32,
                                      qt * 512:(qt + 1) * 512],
                        in_=Os[j][0:32, :])
                dsb = sm.tile([32, 64], F32, tag="dsb", name="dsb")
                nc.sync.dma_start(
                    out=dsb[:], in_=dall[:].rearrange("p a b -> p (a b)"))
                rsb = sm.tile([32, 64], F32, tag="rsb", name="rsb")
                nc.vector.reciprocal(out=rsb[:], in_=dsb[:])
                rdr = dp.tile([1, 4 * 512], F32, name="rdr")
                nc.sync.dma_start(
                    out=bass.AP(tensor=rdr.tensor, offset=rdr.offset,
                                ap=[[64, 32], [1, 64]]),
                    in_=rsb[:])
                rball = sm.tile([P, 512], F32, tag="rball", name="rball")
                nc.scalar.dma_start(
                    out=rball[:],
                    in_=bass.AP(tensor=rdr.tensor, offset=rdr.offset,
                                ap=[[512, 4], [0, 32], [1, 512]]))
                oc = OcatT[hg][:, qt * 512:(qt + 1) * 512]
                nc.vector.tensor_mul(out=oc, in0=oc, in1=rball[:])

            def emit_oproj(qt):
                """O projection + residual for one qt half."""
                for fo in range(2):
                    po = stp.tile([P, 2, 512], F32, tag="ST",
                                  name="po")[:, 0, :]
                    for cc in range(2):
                        nc.tensor.matmul(
                            po[:],
                            lhsT=Ws["WoT"][cc][:, fo * P:(fo + 1) * P],
                            rhs=OcatT[cc][:, qt * 512:(qt + 1) * 512],
                            start=(cc == 0), stop=(cc == 1))
                    nc.vector.scalar_tensor_tensor(
                        out=H1T[fo][:, qt * 512:(qt + 1) * 512], in0=po[:],
                        scalar=pp["bo"][fo],
                        in1=XTs[fo][:, qt * 512:(qt + 1) * 512],
                        op0=ALU.add, op1=ALU.add)

            def cc_reduce(ss, tagp):
                csb = sm.tile([1, P], F32, tag=f"csb_{tagp}",
                              name=f"csb_{tagp}")
                nc.vector.memset(csb[:], 0.0)
                nc.vector.tensor_copy(out=csb[:, 0:2], in_=ss[:])
                cin = dp.tile([1, P], F32)
                cout = dp.tile([1, P], F32)
                nc.gpsimd.dma_start(out=cin[:], in_=csb[:])
                nc.gpsimd.collective_compute(
                    "AllReduce", ALU.add,
                    replica_groups=[[0, 1], [2, 3], [4, 5], [6, 7]],
                    ins=[cin.opt()], outs=[cout.opt()])
                red = sm.tile([1, 2], F32, tag=f"ccr_{tagp}",
                              name=f"ccr_{tagp}")
                nc.gpsimd.dma_start(out=red[:], in_=cout[0:1, 0:2])
                return red

            emit_phase(0, 0)
            emit_phase(0, 1)
            emit_phase(1, 0)
            # O-proj(qt0) deferred into the middle of phase (1,1): by then
            # phase (1,0)'s Ocat rescale chain is done, so the shared ST
            # ring never stalls on it (a >3.4us PE gap cools the HAM clock)
            emit_phase(1, 1, mid=lambda: emit_oproj(0))
            emit_oproj(1)

            # ---- final setnorm (cross-core) + relu + res projection ----
            for i in range(30):
                pw = stp.tile([P, 2, 512], F32, tag="ST", name="warm2")
                nc.tensor.matmul(pw[:, 0, :], lhsT=warm_sb[:, 0:P],
                                 rhs=warm_sb[:], start=True, stop=True)
            ssH = sums_of(H1T, NQ, "h")
            red = cc_reduce(ssH, "h")
            bcH = finish_stats(red, 4 * P, "h")
            fH = factors(bcH, "n0w", "n0b", "h")
            RT = []
            for i in range(2):
                t = pe.tile([P, NQ], BF16, tag=f"RT{i}", name=f"RT{i}")
                nc.scalar.activation(out=t[:], in_=H1T[i][:], func=AF.Relu,
                                     bias=fH[i][1][:], scale=fH[i][0][:])
                RT.append(t)
            OutT = [pe.tile([P, NQ], F32, tag=f"OutT{i}", name=f"OutT{i}")
                    for i in range(2)]
            for qt in range(2):
                for fo in range(2):
                    pr = stp.tile([P, 2, 512], F32, tag="ST", name="pstat")[:, 0, :]
                    for cc in range(2):
                        nc.tensor.matmul(
                            pr[:],
                            lhsT=Ws["WresT"][cc][:, fo * P:(fo + 1) * P],
                            rhs=RT[cc][:, qt * 512:(qt + 1) * 512],
                            start=(cc == 0), stop=(cc == 1))
                    nc.vector.scalar_tensor_tensor(
                        out=OutT[fo][:, qt * 512:(qt + 1) * 512], in0=pr[:],
                        scalar=pp["bres"][fo],
                        in1=H1T[fo][:, qt * 512:(qt + 1) * 512],
                        op0=ALU.add, op1=ALU.add)
                    nc.scalar.dma_start(
                        out=OUT[fo * P:(fo + 1) * P,
                                qt * 512:(qt + 1) * 512],
                        in_=OutT[fo][:, qt * 512:(qt + 1) * 512])

    nc.compile()
    return nc


def _prep_inputs(X, Y, Wq, bq, Wk, bk, Wv, bv, Wo, bo, Wres, bres,
                 nq_w, nq_b, nk_w, nk_b, n0_w, n0_b):
    c = np.ascontiguousarray
    import ml_dtypes
    bf = ml_dtypes.bfloat16
    wcat = np.concatenate([Wv.T, Wk.T, Wq.T, Wo.T, Wres.T],
                          axis=1).astype(bf)
    pv = {"bq": bq, "bk": bk, "bo": bo, "bres": bres, "nqw": nq_w,
          "nqb": nq_b, "nkw": nk_w, "nkb": nk_b, "n0w": n0_w, "n0b": n0_b}
    pvec = np.stack([pv[n] for n in PNAMES], axis=1).astype(np.float32)
    shared = {
        "WCAT": c(wcat),
        "PVEC": c(pvec),
        "bv": c(bv.astype(np.float32)),
    }
    in_maps = []
    for core in range(8):
        b, half = core // 2, core % 2
        Xb = np.asarray(X[b], dtype=np.float32)
        perm = np.concatenate(
            [Xb[half * NQ:(half + 1) * NQ], Xb[(1 - half) * NQ:
                                               (2 - half) * NQ]], axis=0)
        m = dict(shared)
        m["XT"] = c(perm.T)
        m["YT"] = c(np.asarray(Y[b], dtype=np.float32).T)
        in_maps.append(m)
    return in_maps


def run(in_maps, trace=False):
    if "nc" not in _CACHE:
        _CACHE["nc"] = build_module()
    return run_bass_kernel_spmd(_CACHE["nc"], in_maps,
                                core_ids=list(range(8)), trace=trace)


def kernel(**inputs):
    in_maps = _prep_inputs(**inputs)
    res = run(in_maps, trace=False)
    B = 4
    out = np.empty((B, 2 * NQ, D), dtype=np.float32)
    for core in range(8):
        b, half = core // 2, core % 2
        out[b, half * NQ:(half + 1) * NQ, :] = res.results[core]["OUT"].T
    return out


# revision 25
# speedup vs baseline: 1.0809x; 1.0809x over previous
"""Trainium2 Bass kernel for MABClean (cross-attention block with SetNorm).

Sharding: 8 cores = (batch b in 0..3) x (query-half in 0..1). Each core:
  - gets X[b] (rows permuted so its query half comes first) and Y[b], both
    transposed to feature-major [256, 2048] layout,
  - computes SetNorm stats of X/Y on-device, Q for its 1024 queries, full
    K/V, attention (per-head scores via 32-row partition-sliced bf16
    matmuls, exp on ACT emitting fp8, AV as fp8 DoubleRow matmuls packing
    kc-pairs, with a ones-column producing softmax denominators),
    O/residual,
  - AllReduces the final SetNorm (sum, sumsq) with its pair core (tiny
    payload, ring pre-warmed by a dummy collective), applies
    norm+relu+res projection, returns H^T half.
"""

import math

import numpy as np

import concourse.bass as bass
import concourse.tile as tile
from concourse import bacc, mybir
from concourse.bass_utils import run_bass_kernel_spmd

F32 = mybir.dt.float32
BF16 = mybir.dt.bfloat16
FP8 = mybir.dt.float8e4
AF = mybir.ActivationFunctionType
ALU = mybir.AluOpType
PM = mybir.MatmulPerfMode

P = 128
D = 256      # feature dim (dX = dY)
NQ = 1024    # queries per core
NK = 2048    # keys
H = 8        # heads
DH = 32      # head dim
NKC = NK // P    # 16 key chunks
NT = NKC // 2    # 8 kc-pairs for DoubleRow AV
HB = 48          # head block in VO: 32 V dims + 1 ones + 15 pad
EPS = 1e-5
WNAMES = ["WvT", "WkT", "WqT", "WoT", "WresT"]
PNAMES = ["bq", "bk", "bo", "bres", "nqw", "nqb", "nkw", "nkb",
          "n0w", "n0b"]

_CACHE = {}


def build_module():
    nc = bacc.Bacc("TRN2", target_bir_lowering=False, debug=False,
                   num_devices=8)

    # ---- DRAM I/O ----
    XT = nc.dram_tensor("XT", [D, NK], F32, kind="ExternalInput").ap()
    YT = nc.dram_tensor("YT", [D, NK], F32, kind="ExternalInput").ap()
    # all five weight matrices concatenated: [D, 5*D] bf16
    WCAT = nc.dram_tensor("WCAT", [D, 5 * D], BF16, kind="ExternalInput").ap()
    # param vectors concatenated: [D, 10] fp32
    PVEC = nc.dram_tensor("PVEC", [D, len(PNAMES)], F32,
                          kind="ExternalInput").ap()
    bv = nc.dram_tensor("bv", [D], F32, kind="ExternalInput")
    OUT = nc.dram_tensor("OUT", [D, NQ], F32, kind="ExternalOutput").ap()

    with tile.TileContext(nc) as tc:
        with (
            tc.tile_pool(name="persist", bufs=1) as pe,
            tc.tile_pool(name="work", bufs=3) as wk,
            tc.tile_pool(name="small", bufs=4) as sm,
            tc.tile_pool(name="stpool", bufs=2, space="PSUM") as stp,
            tc.tile_pool(name="opool", bufs=1, space="PSUM") as op,
            tc.tile_pool(name="dram", bufs=2, space="DRAM") as dp,
        ):
            # ---- constants / zero zones (gpsimd queue: frees DVE) ----
            ones_col = pe.tile([P, 1], F32, tag="ones_col", name="ones_col")
            nc.gpsimd.memset(ones_col[:], 1.0)
            ones_row = pe.tile([1, P], F32, tag="ones_row", name="ones_row")
            nc.gpsimd.memset(ones_row[:], 1.0)
            zero_col = pe.tile([P, 1], F32, tag="zero_col", name="zero_col")
            nc.gpsimd.memset(zero_col[:], 0.0)
            eps_t = sm.tile([1, 1], F32, tag="eps_t", name="eps_t")
            nc.vector.memset(eps_t[:], EPS)

            # VO: [P, kc-pair, kc-parity, 8 heads x 48]; 32 V dims + ones
            # col (softmax denominator) + 15 zero pad per head block.
            VO = pe.tile([P, NT, 2, H * HB], FP8, tag="VO", name="VO")
            vview = VO[:].rearrange("p t s (h e) -> p t s h e", e=HB)
            nc.gpsimd.memset(vview[:, :, :, :, 32:HB], 0.0)
            nc.gpsimd.memset(vview[:, :, :, :, 32:33], 1.0)

            # ---- load inputs (Y first: its dependency chain is longest) ----
            YTs = [pe.tile([P, NK], F32, tag=f"YT{i}", name=f"YT{i}")
                   for i in range(2)]
            XTs = [pe.tile([P, NK], F32, tag=f"XT{i}", name=f"XT{i}")
                   for i in range(2)]
            qs = [nc.sync, nc.scalar, nc.gpsimd]
            qi = 0
            for h_ in range(4):
                for i in range(2):
                    qs[qi % 3].dma_start(
                        out=YTs[i][:, h_ * 512:(h_ + 1) * 512],
                        in_=YT[i * P:(i + 1) * P, h_ * 512:(h_ + 1) * 512])
                    qi += 1
            wcat = [pe.tile([P, 5 * D], BF16, tag=f"wcat{i}", name=f"wcat{i}")
                    for i in range(2)]
            for i in range(2):
                qs[qi % 3].dma_start(out=wcat[i][:],
                                     in_=WCAT[i * P:(i + 1) * P, :])
                qi += 1
            Ws = {n: [wcat[i][:, k * D:(k + 1) * D] for i in range(2)]
                  for k, n in enumerate(WNAMES)}
            pvt = [pe.tile([P, len(PNAMES)], F32, tag=f"pv{i}", name=f"pv{i}")
                   for i in range(2)]
            for i in range(2):
                qs[qi % 3].dma_start(out=pvt[i][:],
                                     in_=PVEC[i * P:(i + 1) * P, :])
                qi += 1
            pp = {n: [pvt[i][:, k:k + 1] for i in range(2)]
                  for k, n in enumerate(PNAMES)}
            bv_bc = pe.tile([P, D], F32, tag="bv_bc", name="bv_bc")
            nc.sync.dma_start(
                out=bv_bc[:],
                in_=bass.AP(tensor=bv, offset=0, ap=[[0, P], [1, D]]))
            for h_ in range(4):
                for i in range(2):
                    qs[qi % 3].dma_start(
                        out=XTs[i][:, h_ * 512:(h_ + 1) * 512],
                        in_=XT[i * P:(i + 1) * P, h_ * 512:(h_ + 1) * 512])
                    qi += 1

            # ---- PE warm-up: keep the HAM clocked up until real work ----
            warm_sb = pe.tile([P, 512], BF16, tag="warm_sb", name="warm_sb")
            nc.vector.memset(warm_sb[:], 0.0)
            for i in range(52):
                pw = op.tile([P, 512], F32, tag="Os0", name="warm")
                nc.tensor.matmul(pw[:], lhsT=warm_sb[:, 0:P],
                                 rhs=warm_sb[:], start=True, stop=True)

            # ---- pre-warm the collective ring with a dummy AllReduce ----
            cc_sb = sm.tile([1, P], F32, tag="cc_sb", name="cc_sb")
            nc.vector.memset(cc_sb[:], 0.0)
            warm_in = dp.tile([1, P], F32)
            warm_out = dp.tile([1, P], F32)
            nc.gpsimd.dma_start(out=warm_in[:], in_=cc_sb[:])
            nc.gpsimd.collective_compute(
                "AllReduce", ALU.add,
                replica_groups=[[0, 1], [2, 3], [4, 5], [6, 7]],
                ins=[warm_in.opt()], outs=[warm_out.opt()])

            # ---- helpers ----
            def sums_of(chunks, F, tagp):
                """chunks: [128, F] fp32 tiles -> ssum [1,2] SBUF
                (sum of per-partition means, sum of per-partition ex2)."""
                nsub = F // 512
                psum_s = stp.tile([P, 2, 512], F32, tag="ST",
                                  name="pstat")[:, 0, :]
                for ci, ch in enumerate(chunks):
                    sview = ch[:].rearrange("p (n f) -> p n f", f=512)
                    st = sm.tile([P, nsub, 6], F32, tag=f"bns_{tagp}",
                                 name=f"bns_{tagp}")
                    for i in range(nsub):
                        nc.vector.bn_stats(out=st[:, i, :], in_=sview[:, i, :])
                    mv = sm.tile([P, 2], F32, tag=f"mv_{tagp}", name=f"mv_{tagp}")
                    nc.vector.bn_aggr(out=mv[:], in_=st[:])
                    ms2 = sm.tile([P, 2], F32, tag=f"ms2_{tagp}",
                                  name=f"ms2_{tagp}")
                    nc.vector.tensor_copy(out=ms2[:, 0:1], in_=mv[:, 0:1])
                    nc.vector.scalar_tensor_tensor(
                        out=ms2[:, 1:2], in0=mv[:, 0:1], scalar=mv[:, 0:1],
                        in1=mv[:, 1:2], op0=ALU.mult, op1=ALU.add)
                    nc.tensor.matmul(psum_s[0:1, 0:2], lhsT=ones_col[:],
                                     rhs=ms2[:], start=(ci == 0),
                                     stop=(ci == len(chunks) - 1))
                ssum = sm.tile([1, 2], F32, tag=f"ssum_{tagp}",
                               name=f"ssum_{tagp}")
                nc.vector.tensor_copy(out=ssum[:], in_=psum_s[0:1, 0:2])
                return ssum

            def finish_stats(ssum, nparts, tagp):
                """ssum [1,2] -> bc [128,2]: col0 -mean, col1 1/sd."""
                st2 = sm.tile([1, 2], F32, tag=f"st2_{tagp}", name=f"st2_{tagp}")
                nc.vector.tensor_scalar_mul(out=st2[:], in0=ssum[:],
                                            scalar1=1.0 / nparts)
                negvar = sm.tile([1, 1], F32, tag=f"nv_{tagp}", name=f"nv_{tagp}")
                nc.vector.scalar_tensor_tensor(
                    out=negvar[:], in0=st2[:, 0:1], scalar=st2[:, 0:1],
                    in1=st2[:, 1:2], op0=ALU.mult, op1=ALU.subtract)
                sd = sm.tile([1, 1], F32, tag=f"sd_{tagp}", name=f"sd_{tagp}")
                nc.scalar.activation(out=sd[:], in_=negvar[:], func=AF.Sqrt,
                                     bias=eps_t[:], scale=-1.0)
                inv = sm.tile([1, 2], F32, tag=f"inv_{tagp}", name=f"inv_{tagp}")
                nc.vector.reciprocal(out=inv[:, 1:2], in_=sd[:])
                nc.vector.tensor_scalar_mul(out=inv[:, 0:1], in0=st2[:, 0:1],
                                            scalar1=-1.0)
                pb = stp.tile([P, 2, 512], F32, tag="ST",
                              name=f"pb_{tagp}")[:, 0, 0:2]
                nc.tensor.matmul(pb, lhsT=ones_row[:], rhs=inv[:],
                                 start=True, stop=True)
                bc = sm.tile([P, 2], F32, tag=f"bc_{tagp}", name=f"bc_{tagp}")
                nc.vector.tensor_copy(out=bc[:], in_=pb)
                return bc

            def factors(bc, wname, bname, tagp):
                """Per-chunk scale a = w*inv, shift b = a*(-mean) + beta."""
                outs = []
                for i in range(2):
                    a = pe.tile([P, 1], F32, tag=f"a_{tagp}{i}", name=f"a_{tagp}{i}")
                    nc.vector.tensor_scalar_mul(out=a[:], in0=pp[wname][i],
                                                scalar1=bc[:, 1:2])
                    b = pe.tile([P, 1], F32, tag=f"b_{tagp}{i}", name=f"b_{tagp}{i}")
                    nc.vector.scalar_tensor_tensor(
                        out=b[:], in0=a[:], scalar=bc[:, 0:1],
                        in1=pp[bname][i], op0=ALU.mult, op1=ALU.add)
                    outs.append((a, b))
                return outs

            # ---- Y/X stats (X chain overlaps K projection), Yn, K ----
            ssY = sums_of(YTs, NK, "y")
            bcY = finish_stats(ssY, 2 * P, "y")
            fY = factors(bcY, "nkw", "nkb", "y")
            ssX = sums_of(XTs, NK, "x")
            bcX = finish_stats(ssX, 2 * P, "x")
            fX = factors(bcX, "nqw", "nqb", "x")
            YnT = []
            for i in range(2):
                t = pe.tile([P, NK], BF16, tag=f"YnT{i}", name=f"YnT{i}")
                for c4 in range(4):
                    nc.vector.tensor_scalar(
                        out=t[:, c4 * 512:(c4 + 1) * 512],
                        in0=YTs[i][:, c4 * 512:(c4 + 1) * 512],
                        scalar1=fY[i][0][:],
                        scalar2=fY[i][1][:], op0=ALU.mult, op1=ALU.add)
                YnT.append(t)
            KTs = [pe.tile([P, NK], BF16, tag=f"KT{i}", name=f"KT{i}")
                   for i in range(2)]
            pu = 0
            for fo in range(2):
                for nt in range(4):
                    pk = op.tile([P, 512], F32, tag=f"Os{pu % 4}",
                                 name="pproj")
                    pu += 1
                    for cc in range(2):
                        nc.tensor.matmul(
                            pk[:],
                            lhsT=Ws["WkT"][cc][:, fo * P:(fo + 1) * P],
                            rhs=YnT[cc][:, nt * 512:(nt + 1) * 512],
                            start=(cc == 0), stop=(cc == 1))
                    nc.vector.tensor_scalar_add(
                        out=KTs[fo][:, nt * 512:(nt + 1) * 512],
                        in0=pk[:], scalar1=pp["bk"][fo])

            # ---- Xn, Q projection ----
            XnT = []
            for i in range(2):
                t = pe.tile([P, NQ], BF16, tag=f"XnT{i}", name=f"XnT{i}")
                nc.vector.tensor_scalar(
                    out=t[:], in0=XTs[i][:, 0:NQ], scalar1=fX[i][0][:],
                    scalar2=fX[i][1][:], op0=ALU.mult, op1=ALU.add)
                XnT.append(t)
            QTs = [pe.tile([P, NQ], BF16, tag=f"QT{i}", name=f"QT{i}")
                   for i in range(2)]
            # zero-padded per-head copies: full-K matmul contracts one
            # head (216ns/call warm; K=32 sliced matmuls run 2x slower)
            QTz = [[pe.tile([P, NQ], BF16, tag=f"QTz{hg}{j}",
                            name=f"QTz{hg}{j}") for j in range(4)]
                   for hg in range(2)]
            for hg in range(2):
                for j in range(4):
                    nc.gpsimd.memset(QTz[hg][j][:], 0.0)
            for fo in range(2):
                for qt in range(2):
                    pq = op.tile([P, 512], F32, tag=f"Os{pu % 4}",
                                 name="pproj")
                    pu += 1
                    for cc in range(2):
                        nc.tensor.matmul(
                            pq[:],
                            lhsT=Ws["WqT"][cc][:, fo * P:(fo + 1) * P],
                            rhs=XnT[cc][:, qt * 512:(qt + 1) * 512],
                            start=(cc == 0), stop=(cc == 1))
                    nc.vector.tensor_scalar_add(
                        out=QTs[fo][:, qt * 512:(qt + 1) * 512],
                        in0=pq[:], scalar1=pp["bq"][fo])
                for j in range(4):
                    nc.vector.tensor_copy(
                        out=QTz[fo][j][32 * j:32 * j + 32, :],
                        in_=QTs[fo][32 * j:32 * j + 32, :])

            # ---- V projection -> VO (emitted per-kc inside phase 0,0) ----
            Yb = []
            for i in range(2):
                t = pe.tile([P, NK], BF16, tag=f"Yb{i}", name=f"Yb{i}")
                nc.vector.tensor_copy(out=t[:], in_=YTs[i][:])
                Yb.append(t)

            def emit_v(kc):
                global_u = kc
                pv = op.tile([P, 512], F32, tag=f"Os{kc % 4}",
                             name="pproj")[:, 0:D]
                for cc in range(2):
                    nc.tensor.matmul(
                        pv, lhsT=Yb[cc][:, kc * P:(kc + 1) * P],
                        rhs=Ws["WvT"][cc][:],
                        start=(cc == 0), stop=(cc == 1))
                nc.vector.tensor_add(
                    out=vview[:, kc // 2, kc % 2, :, 0:32],
                    in0=pv.rearrange("p (h e) -> p h e", e=32),
                    in1=bv_bc[:].rearrange("p (h e) -> p h e", e=32))
            for kc in range(NKC):
                emit_v(kc)

            # ---- attention ----
            OcatT = [pe.tile([P, NQ], BF16, tag=f"Ocat{i}", name=f"Ocat{i}")
                     for i in range(2)]
            H1T = [pe.tile([P, NQ], F32, tag=f"H1T{i}", name=f"H1T{i}")
                   for i in range(2)]

            def emit_phase(qt, hg, mid=None):
                """Scores -> exp -> AV for one (qt, hg); Ocat rescale."""
                Os = [op.tile([P, 512], F32, tag=f"Os{g}", name=f"Os{g}")
                      for g in range(4)]
                for t in range(NT):
                    if t == 4 and mid is not None:
                        mid()
                    for j in range(4):
                        ST = stp.tile([P, 2, 512], F32, tag="ST", name="ST")
                        ET = wk.tile([P, 2, 512], FP8, tag="ET", name="ET")
                        h = 4 * hg + j
                        for s in range(2):
                            kc = 2 * t + s
                            nc.tensor.matmul(
                                ST[:, s, :],
                                lhsT=KTs[hg][:, kc * P:(kc + 1) * P],
                                rhs=QTz[hg][j][:,
                                              qt * 512:(qt + 1) * 512],
                                start=True, stop=True)
                        nc.scalar.activation(out=ET[:], in_=ST[:],
                                             func=AF.Exp,
                                             bias=zero_col[:],
                                             scale=0.0625)
                        nc.tensor.matmul(
                            Os[j][0:HB, :],
                            lhsT=VO[:, t, :, HB * h:HB * h + HB],
                            rhs=ET[:],
                            start=(t == 0), stop=(t == NT - 1),
                            perf_mode=PM.DoubleRow)
                # free Os quickly: denominators + numerators out to SBUF
                dall = sm.tile([1, 4, 512], F32, tag="dall", name="dall")
                for j in range(4):
                    nc.vector.tensor_copy(out=dall[:, j, :],
                                          in_=Os[j][32:33, :])
                    nc.vector.tensor_copy(
                        out=OcatT[hg][32 * j:32 * j + 32,
                                      qt * 512:(qt + 1) * 512],
                        in_=Os[j][0:32, :])
                dsb = sm.tile([32, 64], F32, tag="dsb", name="dsb")
                nc.sync.dma_start(
                    out=dsb[:], in_=dall[:].rearrange("p a b -> p (a b)"))
                rsb = sm.tile([32, 64], F32, tag="rsb", name="rsb")
                nc.vector.reciprocal(out=rsb[:], in_=dsb[:])
                rdr = dp.tile([1, 4 * 512], F32, name="rdr")
                nc.sync.dma_start(
                    out=bass.AP(tensor=rdr.tensor, offset=rdr.offset,
                                ap=[[64, 32], [1, 64]]),
                    in_=rsb[:])
                rball = sm.tile([P, 512], F32, tag="rball", name="rball")
                nc.scalar.dma_start(
                    out=rball[:],
                    in_=bass.AP(tensor=rdr.tensor, offset=rdr.offset,
                                ap=[[512, 4], [0, 32], [1, 512]]))
                oc = OcatT[hg][:, qt * 512:(qt + 1) * 512]
                nc.vector.tensor_mul(out=oc, in0=oc, in1=rball[:])

            def emit_oproj(qt):
                """O projection + residual for one qt half."""
                for fo in range(2):
                    po = stp.tile([P, 2, 512], F32, tag="ST",
                                  name="po")[:, 0, :]
                    for cc in range(2):
                        nc.tensor.matmul(
                            po[:],
                            lhsT=Ws["WoT"][cc][:, fo * P:(fo + 1) * P],
                            rhs=OcatT[cc][:, qt * 512:(qt + 1) * 512],
                            start=(cc == 0), stop=(cc == 1))
                    nc.vector.scalar_tensor_tensor(
                        out=H1T[fo][:, qt * 512:(qt + 1) * 512], in0=po[:],
                        scalar=pp["bo"][fo],
                        in1=XTs[fo][:, qt * 512:(qt + 1) * 512],
                        op0=ALU.add, op1=ALU.add)

            def cc_reduce(ss, tagp):
                csb = sm.tile([1, P], F32, tag=f"csb_{tagp}",
                              name=f"csb_{tagp}")
                nc.vector.memset(csb[:], 0.0)
                nc.vector.tensor_copy(out=csb[:, 0:2], in_=ss[:])
                cin = dp.tile([1, P], F32)
                cout = dp.tile([1, P], F32)
                nc.gpsimd.dma_start(out=cin[:], in_=csb[:])
                nc.gpsimd.collective_compute(
                    "AllReduce", ALU.add,
                    replica_groups=[[0, 1], [2, 3], [4, 5], [6, 7]],
                    ins=[cin.opt()], outs=[cout.opt()])
                red = sm.tile([1, 2], F32, tag=f"ccr_{tagp}",
                              name=f"ccr_{tagp}")
                nc.gpsimd.dma_start(out=red[:], in_=cout[0:1, 0:2])
                return red

            emit_phase(0, 0)
            emit_phase(0, 1)
            emit_phase(1, 0)
            # O-proj(qt0) deferred into the middle of phase (1,1): by then
            # phase (1,0)'s Ocat rescale chain is done, so the shared ST
            # ring never stalls on it (a >3.4us PE gap cools the HAM clock)
            emit_phase(1, 1, mid=lambda: emit_oproj(0))
            emit_oproj(1)

            # ---- final setnorm (cross-core) + relu + res projection ----
            ssH = sums_of(H1T, NQ, "h")
            red = cc_reduce(ssH, "h")
            bcH = finish_stats(red, 4 * P, "h")
            fH = factors(bcH, "n0w", "n0b", "h")
            RT = []
            for i in range(2):
                t = pe.tile([P, NQ], BF16, tag=f"RT{i}", name=f"RT{i}")
                nc.scalar.activation(out=t[:], in_=H1T[i][:], func=AF.Relu,
                                     bias=fH[i][1][:], scale=fH[i][0][:])
                RT.append(t)
            OutT = [pe.tile([P, NQ], F32, tag=f"OutT{i}", name=f"OutT{i}")
                    for i in range(2)]
            for qt in range(2):
                for fo in range(2):
                    pr = stp.tile([P, 2, 512], F32, tag="ST", name="pstat")[:, 0, :]
                    for cc in range(2):
                        nc.tensor.matmul(
                            pr[:],
                            lhsT=Ws["WresT"][cc][:, fo * P:(fo + 1) * P],
                            rhs=RT[cc][:, qt * 512:(qt + 1) * 512],
                            start=(cc == 0), stop=(cc == 1))
                    nc.vector.scalar_tensor_tensor(
                        out=OutT[fo][:, qt * 512:(qt + 1) * 512], in0=pr[:],
                        scalar=pp["bres"][fo],
                        in1=H1T[fo][:, qt * 512:(qt + 1) * 512],
                        op0=ALU.add, op1=ALU.add)
                    nc.scalar.dma_start(
                        out=OUT[fo * P:(fo + 1) * P,
                                qt * 512:(qt + 1) * 512],
                        in_=OutT[fo][:, qt * 512:(qt + 1) * 512])

    nc.compile()
    return nc


def _prep_inputs(X, Y, Wq, bq, Wk, bk, Wv, bv, Wo, bo, Wres, bres,
                 nq_w, nq_b, nk_w, nk_b, n0_w, n0_b):
    c = np.ascontiguousarray
    import ml_dtypes
    bf = ml_dtypes.bfloat16
    wcat = np.concatenate([Wv.T, Wk.T, Wq.T, Wo.T, Wres.T],
                          axis=1).astype(bf)
    pv = {"bq": bq, "bk": bk, "bo": bo, "bres": bres, "nqw": nq_w,
          "nqb": nq_b, "nkw": nk_w, "nkb": nk_b, "n0w": n0_w, "n0b": n0_b}
    pvec = np.stack([pv[n] for n in PNAMES], axis=1).astype(np.float32)
    shared = {
        "WCAT": c(wcat),
        "PVEC": c(pvec),
        "bv": c(bv.astype(np.float32)),
    }
    in_maps = []
    for core in range(8):
        b, half = core // 2, core % 2
        Xb = np.asarray(X[b], dtype=np.float32)
        perm = np.concatenate(
            [Xb[half * NQ:(half + 1) * NQ], Xb[(1 - half) * NQ:
                                               (2 - half) * NQ]], axis=0)
        m = dict(shared)
        m["XT"] = c(perm.T)
        m["YT"] = c(np.asarray(Y[b], dtype=np.float32).T)
        in_maps.append(m)
    return in_maps


def run(in_maps, trace=False):
    if "nc" not in _CACHE:
        _CACHE["nc"] = build_module()
    return run_bass_kernel_spmd(_CACHE["nc"], in_maps,
                                core_ids=list(range(8)), trace=trace)


def kernel(**inputs):
    in_maps = _prep_inputs(**inputs)
    res = run(in_maps, trace=False)
    B = 4
    out = np.empty((B, 2 * NQ, D), dtype=np.float32)
    for core in range(8):
        b, half = core // 2, core % 2
        out[b, half * NQ:(half + 1) * NQ, :] = res.results[core]["OUT"].T
    return out
